# revision 1
# baseline (speedup 1.0000x reference)
"""Trainium2 Bass kernel: 3x3 conv (NCHW 32x256x56x56, 256->256ch, pad 1) with
a host-expanded synthesized weight, data-parallel over 8 NeuronCores.

Conv as implicit GEMM: for each of the 9 kernel taps, a matmul over a
zero-padded (58x58, padded on host) input image held in SBUF with input
channels on partitions, accumulating over 18 matmuls (9 taps x 2 channel
tiles) in PSUM.  fp16 operands (fp32 accumulate) keep the PE at 1 col/cycle
with LDWEIGHTS fully hidden via fast-weight-load; N = 8 rows x 56 cols = 448
per matmul (PSUM-bank limit is 512 fp32).  Input DMAs are band-split and
spread over both HWDGE rings (sync + scalar), and ~75 junk warmup matmuls
keep the HAM clock gate at 8/8 until the first real matmul (~12us in).
Measured: ~218-228us per core (HFU ~86%); PE matmul-stream floor is ~190us.
"""

import numpy as np

# Problem constants (hardcoded per contract; kernel.py must be self-contained)
OOC, OIC, K1, K2 = 64, 64, 3, 3
R0, R1 = 4, 4
N_CORES = 8
BATCH = 32
N_PER_CORE = BATCH // N_CORES  # 4
C = 256
H = W = 56
HP = WP = H + 2  # zero-padded spatial (padding applied on host)
RB = 8           # output rows per matmul chunk -> N = RB*W = 448
NCH = H // RB    # 7 chunks
KT = C // 128    # 2 input-channel tiles
MT = C // 128    # 2 output-channel tiles
POS = K1 * K2    # 9 kernel taps

# Input-image DMA bands (padded-row ranges): first band covers exactly
# chunk 0 so compute starts early; chunk b needs padded rows 8b .. 8b+9.
BANDS = [(0, 10), (10, 16), (26, 16), (42, 16)]

_NC_CACHE = {}
LAST_RESULT = {}  # test.py introspection: last BassKernelResults


def _expand_weight(weight, alphas, betas):
    """W[p0*64+i, p1*64+j, ky, kx] = w[i,j,ky,kx] * a[p0,p1] / (1+exp(w*b[p0,p1]))."""
    w = weight.astype(np.float32)[None, None]            # (1,1,64,64,3,3)
    a = alphas.astype(np.float32).reshape(R0, R1)[:, :, None, None, None, None]
    b = betas.astype(np.float32).reshape(R0, R1)[:, :, None, None, None, None]
    act = w * a / (1.0 + np.exp(w * b))                  # (4,4,64,64,3,3)
    return act.transpose(0, 2, 1, 3, 4, 5).reshape(R0 * OOC, R1 * OIC, K1, K2)


def _host_prep(x, weight, alphas, betas, bias):
    x = np.asarray(x, dtype=np.float32).astype(np.float16)
    xpad = np.pad(x, ((0, 0), (0, 0), (1, 1), (1, 1)))
    Wfull = _expand_weight(np.asarray(weight), np.asarray(alphas),
                           np.asarray(betas))            # (256,256,3,3)
    # lhsT layout: [ci_local(128 partitions), kt, mt, pos, co_local(128)]
    Wt = Wfull.transpose(1, 0, 2, 3).reshape(C, C, POS)  # (ci, co, pos)
    w_arr = np.ascontiguousarray(
        Wt.reshape(KT, 128, MT, 128, POS).transpose(1, 0, 2, 4, 3)
    ).astype(np.float16)
    b_arr = np.ascontiguousarray(
        np.asarray(bias, dtype=np.float32).reshape(MT, 128).T)
    return xpad, w_arr, b_arr


def _build_nc():
    import concourse.mybir as mybir
    import concourse.tile as tile
    from concourse import bacc

    fp32 = mybir.dt.float32
    fp16 = mybir.dt.float16

    nc = bacc.Bacc("TRN2", target_bir_lowering=False, debug=False,
                   num_devices=N_CORES)

    x_d = nc.dram_tensor("x", [N_PER_CORE, C, HP, WP], fp16,
                         kind="ExternalInput")
    w_d = nc.dram_tensor("w", [128, KT, MT, POS, 128], fp16,
                         kind="ExternalInput")
    b_d = nc.dram_tensor("b", [128, MT], fp32, kind="ExternalInput")
    o_d = nc.dram_tensor("out", [N_PER_CORE, C, H, W], fp32,
                         kind="ExternalOutput")

    # Two HWDGE rings: sync carries kt=0 traffic, scalar carries kt=1.
    def ring(kt):
        return nc.sync if kt == 0 else nc.scalar

    with tile.TileContext(nc) as tc:
        with (
            tc.tile_pool(name="const", bufs=1) as const_pool,
            tc.tile_pool(name="xpad", bufs=1) as xp_pool,
            tc.tile_pool(name="ot", bufs=4) as out_pool,
            tc.tile_pool(name="ps", bufs=6, space="PSUM") as psum_pool,
        ):
            w_sb = const_pool.tile([128, KT, MT, POS, 128], fp16,
                                   name="w_sb", tag="w_sb")
            b_sb = const_pool.tile([128, MT], fp32, name="b_sb", tag="b_sb")

            # PE warmup: ~3.4us of junk matmuls on scratch SBUF during the
            # initial DMA wait flips the HAM clock gate to 8/8 before the
            # real stream starts (and costs nothing - PE is idle anyway).
            warm_in = const_pool.tile([128, 128], fp16, name="warm_in",
                                      tag="warm_in")
            warm_ps = psum_pool.tile([128, 64], fp32, name="warm_ps",
                                     tag="warm_ps", bufs=1)
            nc.vector.memset(warm_in[:], 0.0)
            for _ in range(75):
                nc.tensor.matmul(warm_ps[:], warm_in[:], warm_in[:, 0:64])

            # Double-buffered padded input images (pad arrives from host).
            xp = [[xp_pool.tile([128, HP, WP], fp16, name=f"xp{par}_{kt}",
                                tag=f"xp{par}_{kt}")
                   for kt in range(KT)] for par in range(2)]

            xap = x_d.ap()
            oap = o_d.ap()

            def band_dma(n, par, r0, nr):
                for kt in range(KT):
                    ring(kt).dma_start(
                        xp[par][kt][:, r0:r0 + nr, :],
                        xap[n, kt * 128:(kt + 1) * 128, r0:r0 + nr, :])

            # Head ring order: image-0 band0 first (smallest first-MM
            # critical path), then mt0 weights with the first kernel taps
            # (pos 0-2) ahead so MM#1 waits for the fewest bytes, then the
            # rest.  All head DMAs race the PE warmup.
            band_dma(0, 0, *BANDS[0])
            for kt in range(KT):
                ring(kt).dma_start(w_sb[:, kt, 0, 0:3], w_d.ap()[:, kt, 0, 0:3])
            for kt in range(KT):
                ring(kt).dma_start(w_sb[:, kt, 0, 3:POS],
                                   w_d.ap()[:, kt, 0, 3:POS])
            for kt in range(KT):
                ring(kt).dma_start(w_sb[:, kt, 1], w_d.ap()[:, kt, 1])
            nc.scalar.dma_start(b_sb[:], b_d.ap())

            for n in range(N_PER_CORE):
                par = n % 2
                for r0, nr in (BANDS[1:] if n == 0 else BANDS):
                    band_dma(n, par, r0, nr)
                for ch in range(NCH):
                    y0 = ch * RB
                    for mt in range(MT):
                        ps = psum_pool.tile([128, RB, W], fp32,
                                            name="ps", tag="ps")
                        first = True
                        for kt in range(KT):
                            for dy in range(K1):
                                for dx in range(K2):
                                    pos = dy * K2 + dx
                                    last = (kt == KT - 1 and pos == POS - 1)
                                    nc.tensor.matmul(
                                        ps[:, :, :],
                                        w_sb[:, kt, mt, pos, :],
                                        xp[par][kt][:, y0 + dy:y0 + dy + RB,
                                                    dx:dx + W],
                                        start=first, stop=last,
                                    )
                                    first = False
                        ot = out_pool.tile([128, RB, W], fp32,
                                           name="ot", tag="ot")
                        nc.vector.tensor_scalar_add(ot[:], ps[:],
                                                    b_sb[:, mt:mt + 1])
                        ring(mt).dma_start(
                            oap[n, mt * 128:(mt + 1) * 128, y0:y0 + RB, :],
                            ot[:])
    nc.compile()
    return nc


def get_nc():
    if "nc" not in _NC_CACHE:
        _NC_CACHE["nc"] = _build_nc()
    return _NC_CACHE["nc"]


def kernel(x, weight, alphas, betas, bias):
    from concourse.bass_utils import run_bass_kernel_spmd

    xpad, w_arr, b_arr = _host_prep(x, weight, alphas, betas, bias)
    nc = get_nc()
    in_maps = [
        {"x": xpad[i * N_PER_CORE:(i + 1) * N_PER_CORE], "w": w_arr,
         "b": b_arr}
        for i in range(N_CORES)
    ]
    res = run_bass_kernel_spmd(nc, in_maps, core_ids=list(range(N_CORES)))
    LAST_RESULT["res"] = res
    return np.concatenate([r["out"] for r in res.results], axis=0)



# revision 2
# speedup vs baseline: 1.0132x; 1.0132x over previous
"""Trainium2 Bass kernel: 3x3 conv (NCHW 32x256x56x56, 256->256ch, pad 1) with
a host-expanded synthesized weight, data-parallel over 8 NeuronCores.

Conv as implicit GEMM: for each of the 9 kernel taps, a matmul over a
zero-padded (58x58, padded on host) input image held in SBUF with input
channels on partitions, accumulating over 18 matmuls (9 taps x 2 channel
tiles) in PSUM.  fp16 operands (fp32 accumulate) keep the PE at 1 col/cycle
with LDWEIGHTS fully hidden via fast-weight-load; N = 8 rows x 56 cols = 448
per matmul (PSUM-bank limit is 512 fp32).  ~75 junk warmup matmuls flip the
HAM clock gate to 8/8 during the head DMA wait.  Head DMAs are ordered so
the first 18 real matmuls never stall (weights land whole per (kt,mt) ahead
of first use; stalls re-throttle the PE clock and cost double).  Output is
converted to fp16 on the DVE during the bias add (host converts back to
fp32), halving output DMA bytes.  Measured floor: 1008 matmuls x 448 cols
@2.4GHz = ~190us of PE streaming.
"""

import numpy as np

# Problem constants (hardcoded per contract; kernel.py must be self-contained)
OOC, OIC, K1, K2 = 64, 64, 3, 3
R0, R1 = 4, 4
N_CORES = 8
BATCH = 32
N_PER_CORE = BATCH // N_CORES  # 4
C = 256
H = W = 56
HP = WP = H + 2  # zero-padded spatial (padding applied on host)
RB = 8           # output rows per matmul chunk -> N = RB*W = 448
NCH = H // RB    # 7 chunks
KT = C // 128    # 2 input-channel tiles
MT = C // 128    # 2 output-channel tiles
POS = K1 * K2    # 9 kernel taps

_NC_CACHE = {}
LAST_RESULT = {}  # test.py introspection: last BassKernelResults


def _expand_weight(weight, alphas, betas):
    """W[p0*64+i, p1*64+j, ky, kx] = w[i,j,ky,kx] * a[p0,p1] / (1+exp(w*b[p0,p1]))."""
    w = weight.astype(np.float32)[None, None]            # (1,1,64,64,3,3)
    a = alphas.astype(np.float32).reshape(R0, R1)[:, :, None, None, None, None]
    b = betas.astype(np.float32).reshape(R0, R1)[:, :, None, None, None, None]
    act = w * a / (1.0 + np.exp(w * b))                  # (4,4,64,64,3,3)
    return act.transpose(0, 2, 1, 3, 4, 5).reshape(R0 * OOC, R1 * OIC, K1, K2)


def _host_prep(x, weight, alphas, betas, bias):
    x = np.asarray(x, dtype=np.float32).astype(np.float16)
    xpad = np.pad(x, ((0, 0), (0, 0), (1, 1), (1, 1)))
    Wfull = _expand_weight(np.asarray(weight), np.asarray(alphas),
                           np.asarray(betas))            # (256,256,3,3)
    # lhsT layout: [ci_local(128 partitions), kt, mt, pos, co_local(128)]
    Wt = Wfull.transpose(1, 0, 2, 3).reshape(C, C, POS)  # (ci, co, pos)
    w_arr = np.ascontiguousarray(
        Wt.reshape(KT, 128, MT, 128, POS).transpose(1, 0, 2, 4, 3)
    ).astype(np.float16)
    b_arr = np.ascontiguousarray(
        np.asarray(bias, dtype=np.float32).reshape(MT, 128).T)
    return xpad, w_arr, b_arr


def _build_nc():
    import concourse.mybir as mybir
    import concourse.tile as tile
    from concourse import bacc

    fp32 = mybir.dt.float32
    fp16 = mybir.dt.float16

    nc = bacc.Bacc("TRN2", target_bir_lowering=False, debug=False,
                   num_devices=N_CORES)

    x_d = nc.dram_tensor("x", [N_PER_CORE, C, HP, WP], fp16,
                         kind="ExternalInput")
    w_d = nc.dram_tensor("w", [128, KT, MT, POS, 128], fp16,
                         kind="ExternalInput")
    b_d = nc.dram_tensor("b", [128, MT], fp32, kind="ExternalInput")
    o_d = nc.dram_tensor("out", [N_PER_CORE, C, H, W], fp16,
                         kind="ExternalOutput")

    # Two HWDGE rings: sync carries kt=0 input traffic + mt=0 outputs,
    # scalar carries kt=1 inputs + mt=1 outputs.
    def ring(kt):
        return nc.sync if kt == 0 else nc.scalar

    with tile.TileContext(nc) as tc:
        with (
            tc.tile_pool(name="const", bufs=1) as const_pool,
            tc.tile_pool(name="xpad", bufs=1) as xp_pool,
            tc.tile_pool(name="ot", bufs=4) as out_pool,
            tc.tile_pool(name="ps", bufs=6, space="PSUM") as psum_pool,
        ):
            w_sb = const_pool.tile([128, KT, MT, POS, 128], fp16,
                                   name="w_sb", tag="w_sb")
            b_sb = const_pool.tile([128, MT], fp32, name="b_sb", tag="b_sb")

            # PE warmup: ~4us of junk matmuls on scratch SBUF during the
            # initial DMA wait flips the HAM clock gate to 8/8 before the
            # real stream starts (and costs nothing - PE is idle anyway).
            warm_in = const_pool.tile([128, 128], fp16, name="warm_in",
                                      tag="warm_in")
            warm_ps = psum_pool.tile([128, 64], fp32, name="warm_ps",
                                     tag="warm_ps", bufs=1)
            nc.vector.memset(warm_in[:], 0.0)
            for _ in range(75):
                nc.tensor.matmul(warm_ps[:], warm_in[:], warm_in[:, 0:64])

            # Double-buffered padded input images (pad arrives from host).
            xp = [[xp_pool.tile([128, HP, WP], fp16, name=f"xp{par}_{kt}",
                                tag=f"xp{par}_{kt}")
                   for kt in range(KT)] for par in range(2)]

            xap = x_d.ap()
            oap = o_d.ap()
            wap = w_d.ap()

            def xdma(eng, n, par, kt, r0, r1):
                eng.dma_start(xp[par][kt][:, r0:r1, :],
                              xap[n, kt * 128:(kt + 1) * 128, r0:r1, :])

            # Head: completions on a ring serialize at ~1.5-2us each, so the
            # order below guarantees every operand lands ~2us before its
            # first matmul (a mid-stream stall also re-throttles the PE
            # clock for ~3.4us at half rate).  Ring A (sync) resolves the
            # two band-0 gates first; ring B (scalar) feeds weights whole
            # per (kt, mt) so all 9 taps arrive together.
            xdma(nc.sync, 0, 0, 0, 0, 10)                      # band0 kt0
            nc.scalar.dma_start(w_sb[:, 0, 0], wap[:, 0, 0])   # w kt0 mt0
            xdma(nc.sync, 0, 0, 1, 0, 10)                      # band0 kt1
            nc.scalar.dma_start(w_sb[:, 1, 0], wap[:, 1, 0])   # w kt1 mt0
            nc.sync.dma_start(w_sb[:, 0, 1], wap[:, 0, 1])     # w kt0 mt1
            nc.scalar.dma_start(w_sb[:, 1, 1], wap[:, 1, 1])   # w kt1 mt1
            nc.scalar.dma_start(b_sb[:], b_d.ap())             # bias
            xdma(nc.sync, 0, 0, 0, 10, 26)                     # rows 10:26
            xdma(nc.scalar, 0, 0, 1, 10, 26)
            xdma(nc.sync, 0, 0, 0, 26, 58)                     # rows 26:58
            xdma(nc.scalar, 0, 0, 1, 26, 58)

            for n in range(N_PER_CORE):
                par = n % 2
                if n > 0:
                    for kt in range(KT):
                        xdma(ring(kt), n, par, kt, 0, 58)      # whole image
                for ch in range(NCH):
                    y0 = ch * RB
                    for mt in range(MT):
                        ps = psum_pool.tile([128, RB, W], fp32,
                                            name="ps", tag="ps")
                        first = True
                        for kt in range(KT):
                            for dy in range(K1):
                                for dx in range(K2):
                                    pos = dy * K2 + dx
                                    last = (kt == KT - 1 and pos == POS - 1)
                                    nc.tensor.matmul(
                                        ps[:, :, :],
                                        w_sb[:, kt, mt, pos, :],
                                        xp[par][kt][:, y0 + dy:y0 + dy + RB,
                                                    dx:dx + W],
                                        start=first, stop=last,
                                    )
                                    first = False
                        ot = out_pool.tile([128, RB, W], fp16,
                                           name="ot", tag="ot")
                        nc.vector.tensor_scalar_add(ot[:], ps[:],
                                                    b_sb[:, mt:mt + 1])
                        ring(mt).dma_start(
                            oap[n, mt * 128:(mt + 1) * 128, y0:y0 + RB, :],
                            ot[:])
    nc.compile()
    return nc


def get_nc():
    if "nc" not in _NC_CACHE:
        _NC_CACHE["nc"] = _build_nc()
    return _NC_CACHE["nc"]


def kernel(x, weight, alphas, betas, bias):
    from concourse.bass_utils import run_bass_kernel_spmd

    xpad, w_arr, b_arr = _host_prep(x, weight, alphas, betas, bias)
    nc = get_nc()
    in_maps = [
        {"x": xpad[i * N_PER_CORE:(i + 1) * N_PER_CORE], "w": w_arr,
         "b": b_arr}
        for i in range(N_CORES)
    ]
    res = run_bass_kernel_spmd(nc, in_maps, core_ids=list(range(N_CORES)))
    LAST_RESULT["res"] = res
    return np.concatenate([r["out"] for r in res.results],
                          axis=0).astype(np.float32)
